# revision 16
# baseline (speedup 1.0000x reference)
"""Causal attention (B=4, H=16, S=2048, D=64) on 8 TRN2 NeuronCores.

Sharding: B*H = 64 (b,h) pairs -> 8 pairs per core (embarrassingly parallel,
no collectives). Per core, pairs are processed in 4 "duos" (2 pairs at a
time) so the two D=64 score matmuls can be row-packed into the 128x128 PE
array concurrently (tile_position (0,0) and (64,0)).

v2: the exp over the score matrix (the v1 bottleneck: ~155us of Activation
engine time) is split between the Activation engine (true exp) and the
Vector engine (Schraudolph approximation: P = bitcast_bf16(int16(a*s + b)),
one tensor_scalar op per tile). The split is row-consistent -- each query
row's whole softmax row uses one engine -- so the approximation's sawtooth
bias largely cancels in the softmax ratio (measured ~0.7% output rel err).
Score tiles are built per (pair, 2-k-tile) as [128, 1024] PSUM tiles so
each exp is a single wide instruction. Output PSUM->SBUF copies are also
balanced between ACT and DVE by a greedy assignment.

Per pair algorithm (no running max needed: |score/8| <= ~6 so exp is safe):
  S^T[k, q]   = K @ Q^T           (PE, bf16 inputs, fp32 PSUM)
  P^T         = exp(S^T / 8)      (ACT exact / DVE Schraudolph, -> bf16)
  P^T        *= causal mask       (GpSimd affine_select, diagonal tiles)
  outT[d-ext, q] += V_ext^T @ P^T (PE, accumulated in PSUM over k-tiles)
where V_ext = [V | ones], so outT row 64 carries the softmax denominators.
Host divides and transposes back.
"""

import contextlib
import os
import sys

sys.path.insert(0, "/opt/trn_rl_repo")

import numpy as np
import ml_dtypes

from concourse import bass, bacc, tile, mybir
from concourse.bass_utils import run_bass_kernel_spmd

BF16 = mybir.dt.bfloat16
F32 = mybir.dt.float32
I16 = mybir.dt.int16

B, H, S, D = 4, 16, 2048, 64
NCORES = 8
PAIRS_PER_CORE = (B * H) // NCORES  # 8
NDUO = PAIRS_PER_CORE // 2  # 4
NKT = S // 128  # 16 k-tiles of 128
NQC = S // 512  # 4 q-chunks of 512
VW = D + 1  # 65: V with ones column appended

# Schraudolph constants: q is PRE-SCALED on the host by SCH_A so the scores
# arrive as s' = SCH_A * (q.k), and the DVE exp is a SINGLE-op tensor_scalar
# (two-op mult+add runs ~1.55x slower on HW): bits_i16 = trunc(s' + SCH_B),
# bitcast bf16 gives ~exp(s/8).  a = 128*log2(e)/8; b = 127*128 + c, c tuned
# for trunc conversion (DVE truncates toward zero on f32->i16).  The ACT
# path compensates with scale = 0.125/SCH_A.
SCH_A = float(np.float32(128 * np.log2(np.e) * 0.125))
SCH_B = float(np.float32(127 * 128 - 5.0))
ACT_SCALE = float(np.float32(0.125 / np.float32(SCH_A)))

COPYDELAY = os.environ.get("COPYDELAY", "1") == "1"
PSCFG = os.environ.get("PSCFG", "a")  # a: psS=3/psO=2, b: psS=2/psO=4

_graph_cache = {}


def _exp_cols(qc):
    """Columns the exp instruction covers per (pair, kt2) in chunk qc."""
    cols = []
    for t in range(2 * qc + 2):
        j_e = 2 * t - 4 * qc
        off_e = 128 * j_e if j_e > 0 else 0
        cols.append(1024 - off_e)
    return cols


def _build_assignment():
    """Structured, temporally-interleaved assignment: within each (duo, qc)
    pair A's exp goes to ACT and pair B's to DVE, so the two engines run
    concurrently at every pipeline step.  ACT is ~1.25x faster per column,
    so it additionally takes pair B's smallest units (qc==0) and most output
    copies to even the load.

    Returns (exp_eng, copy_eng): dicts keyed (duo, pairidx, qc) -> "act"|"dve".
    """
    expmode = os.environ.get("EXPMODE", "split")  # split|act|dve
    copymode = os.environ.get("COPYMODE", "mix")  # mix|act|dve
    exp_eng, copy_eng = {}, {}
    for duo in range(NDUO):
        for qc in range(NQC):
            if expmode == "split":
                # ACT is ~1.4x faster than DVE per column on HW; DVE takes
                # pair B's two largest chunks (~36% of columns), plus all
                # output copies (DVE is the only other PSUM-capable engine)
                exp_eng[(duo, 0, qc)] = "act"
                exp_eng[(duo, 1, qc)] = "dve"
            else:
                exp_eng[(duo, 0, qc)] = expmode
                exp_eng[(duo, 1, qc)] = expmode
            if copymode == "mix":
                copy_eng[(duo, 0, qc)] = "act"
                copy_eng[(duo, 1, qc)] = "act" if qc >= 2 else "dve"
            else:
                copy_eng[(duo, 0, qc)] = copymode
                copy_eng[(duo, 1, qc)] = copymode
    return exp_eng, copy_eng


EXP_ENG, COPY_ENG = _build_assignment()


def _emit_copy(nc, otp, o_d, item):
    """Emit the PSUM->SBUF output copy + DMA for a finished (duo, p, qc)."""
    duo, p, qc, oX = item
    osb = otp.tile([VW, 512], F32, tag=f"osb{p}")
    if COPY_ENG[(duo, p, qc)] == "act":
        nc.scalar.copy(osb[:], oX[:])
    else:
        nc.vector.tensor_copy(osb[:], oX[:])
    nc.sync.dma_start(o_d[2 * duo + p, qc], osb[:])


def _body(nc, qt_d, kt_d, vx_d, o_d, qkp, vvp, ptp, otp, psS, psO):
    # output copies are emitted ~one q-chunk late so they never head-of-line
    # block the exp work in the strict-FIFO ACT/DVE queues (psO bufs=4 keeps
    # the accumulator banks free meanwhile)
    pending = []
    for duo in range(NDUO):
        qt = qkp.tile([128, S], BF16, tag="qt")
        kt = qkp.tile([128, S], BF16, tag="kt")
        if duo == 0:
            # chunk the cold-start loads so the first QK can begin after the
            # first quarter arrives (later duos prefetch under compute)
            for c in range(4):
                nc.sync.dma_start(kt[:, 512 * c : 512 * c + 512], kt_d[duo, :, 512 * c : 512 * c + 512])
            for c in range(4):
                nc.sync.dma_start(qt[:, 512 * c : 512 * c + 512], qt_d[duo, :, 512 * c : 512 * c + 512])
        else:
            nc.sync.dma_start(qt[:], qt_d[duo])
            nc.sync.dma_start(kt[:], kt_d[duo])
        vxA = vvp.tile([128, NKT * VW], BF16, tag="vxA")
        nc.sync.dma_start(vxA[:], vx_d[2 * duo])
        vxB = vvp.tile([128, NKT * VW], BF16, tag="vxB")
        nc.sync.dma_start(vxB[:], vx_d[2 * duo + 1])

        for qc in range(NQC):
            oA = psO.tile([VW, 512], F32, tag="o")
            oB = psO.tile([VW, 512], F32, tag="o")
            nkt2 = 2 * qc + 2
            for t in range(nkt2):
                kte, kto = 2 * t, 2 * t + 1
                j_e = kte - 4 * qc
                j_o = kto - 4 * qc
                off_e = 128 * j_e if j_e > 0 else 0
                off_o = 128 * j_o if j_o > 0 else 0
                for p, (vx, oX, rlo, rhi, tp) in enumerate(
                    (
                        (vxA, oA, 0, 64, (0, 0)),
                        (vxB, oB, 64, 128, (64, 0)),
                    )
                ):
                    ps = psS.tile([128, 1024], F32, tag="s")
                    # scores: even k-tile -> cols [off_e:512], odd -> cols
                    # [512+off_o:1024]; pairs run concurrently (row-packed)
                    nc.tensor.matmul(
                        ps[:, off_e:512],
                        kt[rlo:rhi, 128 * kte : 128 * kte + 128],
                        qt[rlo:rhi, 512 * qc + off_e : 512 * qc + 512],
                        start=True,
                        stop=True,
                        tile_position=tp,
                    )
                    nc.tensor.matmul(
                        ps[:, 512 + off_o : 1024],
                        kt[rlo:rhi, 128 * kto : 128 * kto + 128],
                        qt[rlo:rhi, 512 * qc + off_o : 512 * qc + 512],
                        start=True,
                        stop=True,
                        tile_position=tp,
                    )
                    pt = ptp.tile([128, 1024], BF16, tag="pt")
                    # one exp instruction per (pair, kt2); cols [512:512+off_o)
                    # are dead (never read by PV) and may hold exp(garbage)
                    if EXP_ENG[(duo, p, qc)] == "act":
                        nc.scalar.activation(
                            pt[:, off_e:1024],
                            ps[:, off_e:1024],
                            mybir.ActivationFunctionType.Exp,
                            scale=ACT_SCALE,
                        )
                    else:
                        nc.vector.tensor_scalar(
                            pt[:, off_e:1024].bitcast(I16),
                            ps[:, off_e:1024],
                            SCH_B,
                            None,
                            op0=mybir.AluOpType.add,
                        )
                    if j_e >= 0:
                        # causal staircase on the two diagonal 128-blocks
                        for base in (off_e, 512 + off_o):
                            nc.gpsimd.affine_select(
                                pt[:, base : base + 128],
                                pt[:, base : base + 128],
                                pattern=[[1, 128]],
                                compare_op=mybir.AluOpType.is_ge,
                                fill=0.0,
                                base=0,
                                channel_multiplier=-1,
                            )
                    # PV accumulate: outT[65, off:512] += V_ext^T @ P^T
                    nc.tensor.matmul(
                        oX[:, off_e:512],
                        vx[:, VW * kte : VW * kte + VW],
                        pt[:, off_e:512],
                        start=(t == 0),
                        stop=False,
                    )
                    nc.tensor.matmul(
                        oX[:, off_o:512],
                        vx[:, VW * kto : VW * kto + VW],
                        pt[:, 512 + off_o : 1024],
                        start=False,
                        stop=(t == nkt2 - 1),
                    )
                if t == 0 and COPYDELAY:
                    # flush the previous chunk's output copies here, behind
                    # this chunk's first exps, so their PV-stop deps are
                    # already met when the FIFO engine queue reaches them
                    while pending:
                        _emit_copy(nc, otp, o_d, pending.pop(0))

            pending.append((duo, 0, qc, oA))
            pending.append((duo, 1, qc, oB))
            if not COPYDELAY:
                while pending:
                    _emit_copy(nc, otp, o_d, pending.pop(0))

    while pending:
        _emit_copy(nc, otp, o_d, pending.pop(0))


def build_graph(repeat=1):
    """repeat>1 wraps the workload in a hardware For_i loop -- used only for
    timing (marginal wall-clock per iteration = device exec time)."""
    if repeat in _graph_cache:
        return _graph_cache[repeat]

    nc = bacc.Bacc("TRN2", target_bir_lowering=False, debug=False)

    qt_d = nc.dram_tensor("qt", [NDUO, 128, S], BF16, kind="ExternalInput")
    kt_d = nc.dram_tensor("kt", [NDUO, 128, S], BF16, kind="ExternalInput")
    vx_d = nc.dram_tensor(
        "vx", [PAIRS_PER_CORE, 128, NKT * VW], BF16, kind="ExternalInput"
    )
    o_d = nc.dram_tensor(
        "o", [PAIRS_PER_CORE, NQC, VW, 512], F32, kind="ExternalOutput"
    )

    with tile.TileContext(nc) as tc:
        with (
            tc.tile_pool(name="qk", bufs=3) as qkp,
            tc.tile_pool(name="vv", bufs=3) as vvp,
            tc.tile_pool(name="pt", bufs=8) as ptp,
            tc.tile_pool(name="ot", bufs=3) as otp,
            tc.tile_pool(
                name="psS", bufs=3 if PSCFG == "a" else 2, space="PSUM"
            ) as psS,
            tc.tile_pool(
                name="psO", bufs=2 if PSCFG == "a" else 4, space="PSUM"
            ) as psO,
        ):
            rep_ctx = (
                tc.For_i(0, repeat, 1, name="rep")
                if repeat > 1
                else contextlib.nullcontext()
            )
            with rep_ctx:
                _body(nc, qt_d, kt_d, vx_d, o_d, qkp, vvp, ptp, otp, psS, psO)

    nc.compile()
    _graph_cache[repeat] = nc
    return nc


def make_in_maps(query, key, value):
    """Shard + pre-layout the full inputs for the 8 cores."""
    bf = ml_dtypes.bfloat16
    q = np.ascontiguousarray(query, np.float32).reshape(B * H, S, D)
    k = np.ascontiguousarray(key, np.float32).reshape(B * H, S, D)
    v = np.ascontiguousarray(value, np.float32).reshape(B * H, S, D)

    in_maps = []
    for c in range(NCORES):
        sl = slice(c * PAIRS_PER_CORE, (c + 1) * PAIRS_PER_CORE)
        qc_ = q[sl]  # [8, S, D]
        kc_ = k[sl]
        vc_ = v[sl]
        # d-major duo stacking: [4, 128, S]; q pre-scaled by SCH_A (see top)
        qt = (qc_ * np.float32(SCH_A)).transpose(0, 2, 1).reshape(NDUO, 128, S).astype(bf)
        kt = kc_.transpose(0, 2, 1).reshape(NDUO, 128, S).astype(bf)
        # v_ext: [8, 128, NKT*65]
        vx = np.concatenate([vc_, np.ones((PAIRS_PER_CORE, S, 1), np.float32)], 2)
        vx = (
            vx.reshape(PAIRS_PER_CORE, NKT, 128, VW)
            .transpose(0, 2, 1, 3)
            .reshape(PAIRS_PER_CORE, 128, NKT * VW)
            .astype(bf)
        )
        in_maps.append(
            {
                "qt": np.ascontiguousarray(qt),
                "kt": np.ascontiguousarray(kt),
                "vx": np.ascontiguousarray(vx),
            }
        )
    return in_maps


def assemble_output(results):
    """results: list (per core) of dicts with 'o' [8, 4, 65, 512] f32."""
    out = np.empty((B * H, S, D), np.float32)
    for c, r in enumerate(results):
        o = np.asarray(r["o"], np.float32)  # [8, 4, 65, 512]
        for p in range(PAIRS_PER_CORE):
            oT = o[p].transpose(1, 0, 2).reshape(VW, S)  # [65, S]
            out[c * PAIRS_PER_CORE + p] = (oT[0:D] / oT[D : D + 1]).T
    return out.reshape(B, H, S, D)


def kernel(key, value, query, mask=None, **_ignored):
    nc = build_graph()
    in_maps = make_in_maps(query, key, value)
    res = run_bass_kernel_spmd(nc, in_maps, core_ids=list(range(NCORES)))
    return assemble_output(res.results)


if __name__ == "__main__":
    build_graph()
    print("graph built ok")


# revision 20
# speedup vs baseline: 1.0372x; 1.0372x over previous
"""Causal attention (B=4, H=16, S=2048, D=64) on 8 TRN2 NeuronCores.

Sharding: B*H = 64 (b,h) pairs -> 8 pairs per core (embarrassingly parallel,
no collectives). Per core, pairs are processed in 4 "duos" (2 pairs at a
time) so the two D=64 score matmuls can be row-packed into the 128x128 PE
array concurrently (tile_position (0,0) and (64,0)).

v2: the exp over the score matrix (the v1 bottleneck: ~155us of Activation
engine time) is split between the Activation engine (true exp) and the
Vector engine (Schraudolph approximation: P = bitcast_bf16(int16(s' + b))
with q pre-scaled on the host so the DVE op is a SINGLE-op tensor_scalar
-- two-op mult+add runs ~1.55x slower on HW). The split is row-consistent
-- each query row's whole softmax row uses one engine -- which keeps the
sawtooth error contribution at ~share * 2% (measured ~1.1% output rel
err vs the 2e-2 gate). Score tiles are built per (pair, 2-k-tile) as
[128, 1024] PSUM tiles so each exp is one wide instruction (big fixed
per-instruction costs on both engines: ~475ns ACT / ~200ns DVE). Output
PSUM->SBUF copies are mostly on ACT but emitted one q-chunk late so they
never head-of-line block exp work in the strict-FIFO engine queues.

Per pair algorithm (no running max needed: |score/8| <= ~6 so exp is safe):
  S^T[k, q]   = K @ Q^T           (PE, bf16 inputs, fp32 PSUM)
  P^T         = exp(S^T / 8)      (ACT exact / DVE Schraudolph, -> bf16)
  P^T        *= causal mask       (GpSimd affine_select, diagonal tiles)
  outT[d-ext, q] += V_ext^T @ P^T (PE, accumulated in PSUM over k-tiles)
where V_ext = [V | ones], so outT row 64 carries the softmax denominators.
Host divides and transposes back.
"""

import contextlib
import os
import sys

sys.path.insert(0, "/opt/trn_rl_repo")

import numpy as np
import ml_dtypes

from concourse import bass, bacc, tile, mybir
from concourse.bass_utils import run_bass_kernel_spmd

BF16 = mybir.dt.bfloat16
F32 = mybir.dt.float32
I16 = mybir.dt.int16

B, H, S, D = 4, 16, 2048, 64
NCORES = 8
PAIRS_PER_CORE = (B * H) // NCORES  # 8
NDUO = PAIRS_PER_CORE // 2  # 4
NKT = S // 128  # 16 k-tiles of 128
NQC = S // 512  # 4 q-chunks of 512
VW = D + 1  # 65: V with ones column appended

# Schraudolph constants: q is PRE-SCALED on the host by SCH_A so the scores
# arrive as s' = SCH_A * (q.k), and the DVE exp is a SINGLE-op tensor_scalar
# (two-op mult+add runs ~1.55x slower on HW): bits_i16 = trunc(s' + SCH_B),
# bitcast bf16 gives ~exp(s/8).  a = 128*log2(e)/8; b = 127*128 + c, c tuned
# for trunc conversion (DVE truncates toward zero on f32->i16).  The ACT
# path compensates with scale = 0.125/SCH_A.
SCH_A = float(np.float32(128 * np.log2(np.e) * 0.125))
SCH_B = float(np.float32(127 * 128 - 5.0))
ACT_SCALE = float(np.float32(0.125 / np.float32(SCH_A)))

COPYDELAY = os.environ.get("COPYDELAY", "1") == "1"
# k-tile grouping per q-chunk: "3" packs up to 3 k-tiles per score tile
# ([128,1536] = 3 PSUM banks, psS bufs=2) halving the count of exp
# instructions (each carries ~475ns ACT / ~200ns DVE fixed cost); "2" is
# the kt2 layout ([128,1024], psS bufs=3). Both keep 6 banks of score
# buffering.
GRPMODE = os.environ.get("GRPMODE", "2")


def _groups(qc):
    nkt = 4 * qc + 4
    if GRPMODE == "2":
        return [2] * (nkt // 2)
    return {4: [2, 2], 8: [3, 3, 2], 12: [3, 3, 3, 3], 16: [3, 3, 3, 3, 2, 2]}[nkt]

_graph_cache = {}


def _exp_cols(qc):
    """Columns the exp instruction covers per (pair, kt2) in chunk qc."""
    cols = []
    for t in range(2 * qc + 2):
        j_e = 2 * t - 4 * qc
        off_e = 128 * j_e if j_e > 0 else 0
        cols.append(1024 - off_e)
    return cols


def _build_assignment():
    """Structured, temporally-interleaved assignment: within each (duo, qc)
    pair A's exp goes to ACT and pair B's to DVE, so the two engines run
    concurrently at every pipeline step.  ACT is ~1.25x faster per column,
    so it additionally takes pair B's smallest units (qc==0) and most output
    copies to even the load.

    Returns (exp_eng, copy_eng): dicts keyed (duo, pairidx, qc) -> "act"|"dve".
    """
    expmode = os.environ.get("EXPMODE", "split")  # split|act|dve
    copymode = os.environ.get("COPYMODE", "mix")  # mix|act|dve
    exp_eng, copy_eng = {}, {}
    for duo in range(NDUO):
        for qc in range(NQC):
            if expmode == "split":
                # ACT is ~1.4x faster than DVE per column on HW; DVE takes
                # pair B's two largest chunks (~36% of columns), plus all
                # output copies (DVE is the only other PSUM-capable engine)
                exp_eng[(duo, 0, qc)] = "act"
                exp_eng[(duo, 1, qc)] = "dve"
            else:
                exp_eng[(duo, 0, qc)] = expmode
                exp_eng[(duo, 1, qc)] = expmode
            if copymode == "mix":
                copy_eng[(duo, 0, qc)] = "act"
                copy_eng[(duo, 1, qc)] = "act" if qc >= 2 else "dve"
            else:
                copy_eng[(duo, 0, qc)] = copymode
                copy_eng[(duo, 1, qc)] = copymode
    return exp_eng, copy_eng


EXP_ENG, COPY_ENG = _build_assignment()


def _emit_copy(nc, otp, o_d, item):
    """Emit the PSUM->SBUF output copy + DMA for a finished (duo, p, qc)."""
    duo, p, qc, oX = item
    osb = otp.tile([VW, 512], F32, tag=f"osb{p}")
    if COPY_ENG[(duo, p, qc)] == "act":
        nc.scalar.copy(osb[:], oX[:])
    else:
        nc.vector.tensor_copy(osb[:], oX[:])
    nc.sync.dma_start(o_d[2 * duo + p, qc], osb[:])


def _body(nc, qt_d, kt_d, vx_d, o_d, qkp, vvp, ptp, otp, psS, psO):
    # output copies are emitted ~one q-chunk late so they never head-of-line
    # block the exp work in the strict-FIFO ACT/DVE queues (psO bufs=4 keeps
    # the accumulator banks free meanwhile)
    pending = []
    for duo in range(NDUO):
        qt = qkp.tile([128, S], BF16, tag="qt")
        kt = qkp.tile([128, S], BF16, tag="kt")
        if duo == 0:
            # chunk the cold-start loads so the first QK can begin after the
            # first quarter arrives (later duos prefetch under compute)
            for c in range(4):
                nc.sync.dma_start(kt[:, 512 * c : 512 * c + 512], kt_d[duo, :, 512 * c : 512 * c + 512])
            for c in range(4):
                nc.sync.dma_start(qt[:, 512 * c : 512 * c + 512], qt_d[duo, :, 512 * c : 512 * c + 512])
        else:
            nc.sync.dma_start(qt[:], qt_d[duo])
            nc.sync.dma_start(kt[:], kt_d[duo])
        vxA = vvp.tile([128, NKT * VW], BF16, tag="vxA")
        nc.sync.dma_start(vxA[:], vx_d[2 * duo])
        vxB = vvp.tile([128, NKT * VW], BF16, tag="vxB")
        nc.sync.dma_start(vxB[:], vx_d[2 * duo + 1])

        for qc in range(NQC):
            oA = psO.tile([VW, 512], F32, tag="o")
            oB = psO.tile([VW, 512], F32, tag="o")
            nkt = 4 * qc + 4
            k0 = 0
            for gi, g in enumerate(_groups(qc)):
                for p, (vx, oX, rlo, rhi, tp) in enumerate(
                    (
                        (vxA, oA, 0, 64, (0, 0)),
                        (vxB, oB, 64, 128, (64, 0)),
                    )
                ):
                    ps = psS.tile([128, 512 * g], F32, tag="s")
                    offs = []
                    for i in range(g):
                        kti = k0 + i
                        j = kti - 4 * qc
                        off = 128 * j if j > 0 else 0
                        offs.append(off)
                        # scores for k-tile kti -> cols [512i+off : 512(i+1)];
                        # pairs run concurrently (row-packed tile_position)
                        nc.tensor.matmul(
                            ps[:, 512 * i + off : 512 * i + 512],
                            kt[rlo:rhi, 128 * kti : 128 * kti + 128],
                            qt[rlo:rhi, 512 * qc + off : 512 * qc + 512],
                            start=True,
                            stop=True,
                            tile_position=tp,
                        )
                    pt = ptp.tile([128, 512 * g], BF16, tag="pt")
                    # one exp instruction per (pair, group); inter-tile dead
                    # spans are never read by the narrowed PV matmuls
                    if EXP_ENG[(duo, p, qc)] == "act":
                        nc.scalar.activation(
                            pt[:, offs[0] : 512 * g],
                            ps[:, offs[0] : 512 * g],
                            mybir.ActivationFunctionType.Exp,
                            scale=ACT_SCALE,
                        )
                    else:
                        nc.vector.tensor_scalar(
                            pt[:, offs[0] : 512 * g].bitcast(I16),
                            ps[:, offs[0] : 512 * g],
                            SCH_B,
                            None,
                            op0=mybir.AluOpType.add,
                        )
                    for i in range(g):
                        kti = k0 + i
                        if kti - 4 * qc >= 0:
                            # causal staircase on the diagonal 128-block
                            base = 512 * i + offs[i]
                            nc.gpsimd.affine_select(
                                pt[:, base : base + 128],
                                pt[:, base : base + 128],
                                pattern=[[1, 128]],
                                compare_op=mybir.AluOpType.is_ge,
                                fill=0.0,
                                base=0,
                                channel_multiplier=-1,
                            )
                    for i in range(g):
                        kti = k0 + i
                        off = offs[i]
                        # PV accumulate: outT[65, off:512] += V_ext^T @ P^T
                        nc.tensor.matmul(
                            oX[:, off:512],
                            vx[:, VW * kti : VW * kti + VW],
                            pt[:, 512 * i + off : 512 * (i + 1)],
                            start=(kti == 0),
                            stop=(kti == nkt - 1),
                        )
                if gi == 0 and COPYDELAY:
                    # flush the previous chunk's output copies here, behind
                    # this chunk's first exps, so their PV-stop deps are
                    # already met when the FIFO engine queue reaches them
                    while pending:
                        _emit_copy(nc, otp, o_d, pending.pop(0))
                k0 += g

            pending.append((duo, 0, qc, oA))
            pending.append((duo, 1, qc, oB))
            if not COPYDELAY:
                while pending:
                    _emit_copy(nc, otp, o_d, pending.pop(0))

    while pending:
        _emit_copy(nc, otp, o_d, pending.pop(0))


def build_graph(repeat=1):
    """repeat>1 wraps the workload in a hardware For_i loop -- used only for
    timing (marginal wall-clock per iteration = device exec time)."""
    if repeat in _graph_cache:
        return _graph_cache[repeat]

    nc = bacc.Bacc("TRN2", target_bir_lowering=False, debug=False)

    qt_d = nc.dram_tensor("qt", [NDUO, 128, S], BF16, kind="ExternalInput")
    kt_d = nc.dram_tensor("kt", [NDUO, 128, S], BF16, kind="ExternalInput")
    vx_d = nc.dram_tensor(
        "vx", [PAIRS_PER_CORE, 128, NKT * VW], BF16, kind="ExternalInput"
    )
    o_d = nc.dram_tensor(
        "o", [PAIRS_PER_CORE, NQC, VW, 512], F32, kind="ExternalOutput"
    )

    with tile.TileContext(nc) as tc:
        with (
            tc.tile_pool(name="qk", bufs=3) as qkp,
            tc.tile_pool(name="vv", bufs=3) as vvp,
            tc.tile_pool(name="pt", bufs=12) as ptp,
            tc.tile_pool(name="ot", bufs=4) as otp,
            tc.tile_pool(
                name="psS", bufs=3 if GRPMODE == "2" else 2, space="PSUM"
            ) as psS,
            tc.tile_pool(name="psO", bufs=2, space="PSUM") as psO,
        ):
            rep_ctx = (
                tc.For_i(0, repeat, 1, name="rep")
                if repeat > 1
                else contextlib.nullcontext()
            )
            with rep_ctx:
                _body(nc, qt_d, kt_d, vx_d, o_d, qkp, vvp, ptp, otp, psS, psO)

    nc.compile()
    _graph_cache[repeat] = nc
    return nc


def make_in_maps(query, key, value):
    """Shard + pre-layout the full inputs for the 8 cores."""
    bf = ml_dtypes.bfloat16
    q = np.ascontiguousarray(query, np.float32).reshape(B * H, S, D)
    k = np.ascontiguousarray(key, np.float32).reshape(B * H, S, D)
    v = np.ascontiguousarray(value, np.float32).reshape(B * H, S, D)

    in_maps = []
    for c in range(NCORES):
        sl = slice(c * PAIRS_PER_CORE, (c + 1) * PAIRS_PER_CORE)
        qc_ = q[sl]  # [8, S, D]
        kc_ = k[sl]
        vc_ = v[sl]
        # d-major duo stacking: [4, 128, S]; q pre-scaled by SCH_A (see top)
        qt = (qc_ * np.float32(SCH_A)).transpose(0, 2, 1).reshape(NDUO, 128, S).astype(bf)
        kt = kc_.transpose(0, 2, 1).reshape(NDUO, 128, S).astype(bf)
        # v_ext: [8, 128, NKT*65]
        vx = np.concatenate([vc_, np.ones((PAIRS_PER_CORE, S, 1), np.float32)], 2)
        vx = (
            vx.reshape(PAIRS_PER_CORE, NKT, 128, VW)
            .transpose(0, 2, 1, 3)
            .reshape(PAIRS_PER_CORE, 128, NKT * VW)
            .astype(bf)
        )
        in_maps.append(
            {
                "qt": np.ascontiguousarray(qt),
                "kt": np.ascontiguousarray(kt),
                "vx": np.ascontiguousarray(vx),
            }
        )
    return in_maps


def assemble_output(results):
    """results: list (per core) of dicts with 'o' [8, 4, 65, 512] f32."""
    out = np.empty((B * H, S, D), np.float32)
    for c, r in enumerate(results):
        o = np.asarray(r["o"], np.float32)  # [8, 4, 65, 512]
        for p in range(PAIRS_PER_CORE):
            oT = o[p].transpose(1, 0, 2).reshape(VW, S)  # [65, S]
            out[c * PAIRS_PER_CORE + p] = (oT[0:D] / oT[D : D + 1]).T
    return out.reshape(B, H, S, D)


def kernel(key, value, query, mask=None, **_ignored):
    nc = build_graph()
    in_maps = make_in_maps(query, key, value)
    res = run_bass_kernel_spmd(nc, in_maps, core_ids=list(range(NCORES)))
    return assemble_output(res.results)


if __name__ == "__main__":
    build_graph()
    print("graph built ok")
